# revision 11
# baseline (speedup 1.0000x reference)
"""Contrastive loss on 8 TRN2 cores — v4 (A2A-pipelined gram, overlapped front).

v3 -> v4: the two big sim ReduceScatters are replaced by five per-chunk
AllToAlls (measured ~2x cheaper incl. floor) with DVE tree-adds, pipelined
under the remaining gram matmuls; x loads in 64 small DMAs so squares (now
ACT-only, r-halves) start during the load; ssq/rsqrt/normalize run in
r-halves so normalize starts ~15us; per-chunk exp/colsum/mask loss work
overlaps the collective chain; positives read pre-exp (no ln(exp));
final denominator combine via a [128,16] A2A; activation table sets
sequenced so no ACT_TABLE_LOAD lands on the critical path.
"""

import numpy as np
import ml_dtypes

import concourse.bacc as bacc
import concourse.mybir as mybir
import concourse.tile as tile
from concourse import bass_utils

F32 = mybir.dt.float32
F8E5 = mybir.dt.float8e5
BF16 = mybir.dt.bfloat16
FP8 = mybir.dt.float8e4
AF = mybir.ActivationFunctionType
ALU = mybir.AluOpType
PM = mybir.MatmulPerfMode

B = 1024
R = 2 * B
NCORES = 8
KT = 8
S = 16.0
INV_T_S2 = 2.0 / (S * S)
CH1 = 1024
CH2 = 1152

# column chunks: (space, lo, hi); A = cc1 cols, B = cc2 cols
CHUNKS = [("A", 0, 512), ("A", 512, 1024),
          ("B", 0, 512), ("B", 512, 1024), ("B", 1024, 1152)]

_CACHE = {}


def _pieces(c, lo, hi):
    """Segment pieces of cc2-local cols [lo,hi) for pair c.

    Returns (psum_off, sim_col, width, tile_row) pieces, 512-split."""
    wa2 = 1024 - 128 * c
    tb = 15 - c
    segs = []
    a0, a1 = max(lo, 0), min(hi, wa2)
    if a1 > a0:
        segs.append((a0 - lo, 128 * c + CH1 + a0, a1 - a0, c))
    b0, b1 = max(lo, wa2), min(hi, CH2)
    if b1 > b0:
        segs.append((b0 - lo, 128 * tb + (b0 - wa2), b1 - b0, tb))
    out = []
    for po, sc, w, tr in segs:
        off = 0
        while off < w:
            ww = min(512, w - off)
            out.append((po + off, sc + off, ww, tr))
            off += ww
    return out


def _chunk_units(space, lo, hi):
    """Per-unit pieces for a column chunk. unit u -> row block 128u of the
    collective payload (A2A shard u -> core u)."""
    units = []
    for u in range(8):
        if space == "A":
            units.append([(0, 128 * u + lo, hi - lo, u)])
        else:
            units.append(_pieces(u, lo, hi))
    return units


def _build_nc():
    if "nc" in _CACHE:
        return _CACHE["nc"]
    nc = bacc.Bacc("TRN2", target_bir_lowering=False, debug=False,
                   num_devices=NCORES)

    x = nc.dram_tensor("x", [KT, 128, 2 * R], FP8, kind="ExternalInput")
    sel = nc.dram_tensor("sel", [128, 256], FP8, kind="ExternalInput")
    eye = nc.dram_tensor("eye", [128, 128], BF16, kind="ExternalInput")
    msk = nc.dram_tensor("msk", [2, 128, CH2], BF16, kind="ExternalInput")
    rt = nc.dram_tensor("rt", [16, 16], BF16, kind="ExternalInput")
    cm = nc.dram_tensor("cm", [2, 128, 16], F32, kind="ExternalInput")
    y = nc.dram_tensor("y", [1, 1], F32, kind="ExternalOutput")

    cc = []
    for ci, (space, lo, hi) in enumerate(CHUNKS):
        w = hi - lo
        cin = nc.dram_tensor(f"cc{ci}_in", [1024, w], F8E5)
        cout = nc.dram_tensor(f"cc{ci}_out", [1024, w], F8E5)
        cc.append((cin, cout))
    ccd_in = nc.dram_tensor("ccd_in", [128, 16], F32)
    ccd_out = nc.dram_tensor("ccd_out", [128, 16], F32)
    wu_in = nc.dram_tensor("wu_in", [128, 16], F8E5)
    wu_out = nc.dram_tensor("wu_out", [128, 16], F8E5)
    grp = [list(range(NCORES))]

    with tile.TileContext(nc) as tc:
        with tc.tile_pool(name="x8", bufs=KT) as px8, \
             tc.tile_pool(name="sq", bufs=3) as psq, \
             tc.tile_pool(name="pers", bufs=1) as pers, \
             tc.tile_pool(name="simsb", bufs=16) as psim, \
             tc.tile_pool(name="acc", bufs=10) as pacc, \
             tc.tile_pool(name="sum", bufs=8) as psum_pool, \
             tc.tile_pool(name="loss", bufs=1) as plo, \
             tc.tile_pool(name="ps", bufs=8, space="PSUM") as pps:

            # ---- x load: 8 sub-DMAs per k-tile for queue stagger ----
            xb = []
            for k in range(KT):
                t = px8.tile([128, 2 * R], FP8, tag="x8")
                for i in range(8):
                    nc.sync.dma_start(t[16 * i:16 * (i + 1), :],
                                      x[k, 16 * i:16 * (i + 1), :])
                xb.append(t)

            # ACT warm: force abs_reciprocal_sqrt_and_small (contains
            # square) before the first data-gated Square runs.
            junk = pers.tile([128, 16], F32, tag="junk")
            nc.vector.memset(junk[:], 1.0)
            junk2 = pers.tile([128, 16], F32, tag="junk2")
            nc.scalar.activation(junk2[:], junk[:], AF.Abs_reciprocal_sqrt)

            selb = pers.tile([128, 256], FP8, tag="selb")
            nc.sync.dma_start(selb[:], sel[:])
            eyeb = pers.tile([128, 128], BF16, tag="eyeb")
            nc.sync.dma_start(eyeb[:], eye[:])
            mskb = pers.tile([128, 2 * CH2], BF16, tag="mskb")
            nc.sync.dma_start(mskb[:, 0:CH2], msk[0, :, :])
            nc.sync.dma_start(mskb[:, CH2:2 * CH2], msk[1, :, :])
            rtb = pers.tile([16, 16], BF16, tag="rtb")
            nc.sync.dma_start(rtb[:], rt[:])
            cmb = pers.tile([128, 32], F32, tag="cmb")
            nc.sync.dma_start(cmb[:, 0:16], cm[0, :, :])
            nc.sync.dma_start(cmb[:, 16:32], cm[1, :, :])
            ones1 = pers.tile([128, 1], BF16, tag="ones1")
            nc.vector.memset(ones1[:], 1.0)
            onesf = pers.tile([128, 1], F32, tag="onesf")
            nc.vector.memset(onesf[:], 1.0)
            negf2 = pers.tile([128, 1], F32, tag="negf2")
            nc.vector.memset(negf2[:], -2.0 * INV_T_S2)

            # warmup collective (absorbs first-call setup)
            wub = pers.tile([128, 16], F8E5, tag="wub")
            nc.vector.memset(wub[:], 1.0)
            nc.sync.dma_start(wu_in[:], wub[:])
            nc.gpsimd.collective_compute(
                "AllToAll", ALU.bypass, replica_groups=grp,
                ins=[wu_in[:].opt()], outs=[wu_out[:].opt()])

            selv = selb[:].rearrange("p (two j) -> p two j", two=2)
            scale_t = pers.tile([128, R], FP8, tag="scale_t")

            def vk(k):
                return xb[k][:].rearrange("p (two r) -> p two r", two=2)

            # ---- half h=0: squares (ACT) + ssq matmuls ----
            def do_squares(h):
                ssq = [pps.tile([128, 512], F32, tag="ps", name=f"ssq{h}{j}")
                       for j in range(2)]
                for k in range(KT):
                    sq = psq.tile([128, 2048], FP8, tag="sq")
                    sqv = sq[:].rearrange("p (two r) -> p two r", two=2)
                    nc.scalar.activation(
                        sqv, vk(k)[:, :, 1024 * h:1024 * (h + 1)], AF.Square)
                    for j in range(2):
                        nc.tensor.matmul(ssq[j][:], selv,
                                         sqv[:, :, 512 * j:512 * (j + 1)],
                                         start=(k == 0), stop=(k == KT - 1),
                                         perf_mode=PM.DoubleRow)
                for j in range(2):
                    off = 1024 * h + 512 * j
                    nc.scalar.activation(scale_t[:, off:off + 512],
                                         ssq[j][:], AF.Abs_reciprocal_sqrt,
                                         scale=128.0 / (S * S))

            def do_norm(h, k):
                for s in range(2):
                    off = s * R + 1024 * h
                    nc.vector.tensor_tensor(
                        xb[k][:, off:off + 1024], xb[k][:, off:off + 1024],
                        scale_t[:, 1024 * h:1024 * h + 1024], ALU.mult)

            do_squares(0)

            # ---- half h=1 squares interleaved with norm h=0 + early gram
            ssq1 = [pps.tile([128, 512], F32, tag="ps", name=f"ssq1{j}")
                    for j in range(2)]

            # gram chunk machinery -------------------------------------
            chunk_tiles = {}   # ci -> list of (psum_tile, width)

            def gram_half(ci, half, ks, unit_major=True):
                """Issue matmuls for units [4*half,4*half+4) of chunk ci
                over k list ks. start/stop track global k bounds."""
                space, lo, hi = CHUNKS[ci]
                w = hi - lo
                units = _chunk_units(space, lo, hi)
                if ci not in chunk_tiles:
                    chunk_tiles[ci] = [None] * 8
                for ui in range(4 * half, 4 * half + 4):
                    if chunk_tiles[ci][ui] is None:
                        chunk_tiles[ci][ui] = pps.tile(
                            [128, w], F32, tag="ps", name=f"g{ci}_{ui}")
                uis = list(range(4 * half, 4 * half + 4))
                order = ([(ui, k) for ui in uis for k in ks] if unit_major
                         else [(ui, k) for k in ks for ui in uis])
                for ui, k in order:
                    v = vk(k)
                    pt = chunk_tiles[ci][ui]
                    for po, sc, ww, tr in units[ui]:
                        lhsT = v[:, :, 128 * tr:128 * (tr + 1)]
                        nc.tensor.matmul(pt[:, po:po + ww], lhsT,
                                         v[:, :, sc:sc + ww],
                                         start=(k == 0),
                                         stop=(k == KT - 1),
                                         perf_mode=PM.DoubleRow)

            def ship_chunk(ci):
                """Copy chunk psums to SBUF fp8e5, DMA to cc_in, A2A."""
                space, lo, hi = CHUNKS[ci]
                w = hi - lo
                cin, cout = cc[ci]
                for ui in range(8):
                    pt = chunk_tiles[ci][ui]
                    sb = psim.tile([128, w], F8E5, tag="simsb")
                    nc.scalar.activation(sb[:], pt[:], AF.Copy)
                    nc.sync.dma_start(cin[128 * ui:128 * ui + 64, :],
                                      sb[0:64, :])
                    nc.sync.dma_start(cin[128 * ui + 64:128 * (ui + 1), :],
                                      sb[64:128, :])
                nc.gpsimd.collective_compute(
                    "AllToAll", ALU.bypass, replica_groups=grp,
                    ins=[cin[:].opt()], outs=[cout[:].opt()])

            # interleave: per k -> h1 square, ssq1 mms, norm-h0, gram A0a
            for k in range(KT):
                sq = psq.tile([128, 2048], FP8, tag="sq")
                sqv = sq[:].rearrange("p (two r) -> p two r", two=2)
                nc.scalar.activation(sqv, vk(k)[:, :, 1024:2048], AF.Square)
                for j in range(2):
                    nc.tensor.matmul(ssq1[j][:], selv,
                                     sqv[:, :, 512 * j:512 * (j + 1)],
                                     start=(k == 0), stop=(k == KT - 1),
                                     perf_mode=PM.DoubleRow)
                do_norm(0, k)
                gram_half(0, 0, [k], unit_major=False)
            for j in range(2):
                off = 1024 + 512 * j
                nc.scalar.activation(scale_t[:, off:off + 512],
                                     ssq1[j][:], AF.Abs_reciprocal_sqrt,
                                     scale=128.0 / (S * S))

            # norm h=1 + remaining gram
            for k in range(KT):
                do_norm(1, k)
            gram_half(0, 1, list(range(KT)))
            ship_chunk(0)
            for ci in range(1, len(CHUNKS)):
                gram_half(ci, 0, list(range(KT)))
                gram_half(ci, 1, list(range(KT)))
                ship_chunk(ci)

            # ---- per-chunk post-collective loss work ----
            # accumulators
            rs_parts = []      # exp rowsum per chunk [128,1]
            rsA2_parts = []    # cc2 "tile-c side" masked exp rowsums
            expdB_parts = []
            expdA = None
            possum = None
            pc_sb = plo.tile([128, 16], BF16, tag="pc_sb")
            pc_idx = 0

            for ci, (space, lo, hi) in enumerate(CHUNKS):
                w = hi - lo
                cin, cout = cc[ci]
                ld = psum_pool.tile([128, 8 * w], F8E5, tag="ld",
                                    name=f"ld{ci}", bufs=2)
                for s in range(8):
                    nc.sync.dma_start(ld[:, s * w:(s + 1) * w],
                                      cout[128 * s:128 * (s + 1), :])
                # tree add 8 partials -> bf16 sim
                lv = ld[:].rearrange("p (s w) -> p s w", s=8)
                t4 = []
                for a in range(4):
                    tt = psum_pool.tile([128, w], BF16, tag="t4",
                                        name=f"t4_{ci}_{a}", bufs=4)
                    nc.vector.tensor_tensor(tt[:], lv[:, 2 * a, :],
                                            lv[:, 2 * a + 1, :], ALU.add)
                    t4.append(tt)
                t2 = []
                for a in range(2):
                    tt = psum_pool.tile([128, w], BF16, tag="t2",
                                        name=f"t2_{ci}_{a}", bufs=2)
                    nc.vector.tensor_tensor(tt[:], t4[2 * a][:],
                                            t4[2 * a + 1][:], ALU.add)
                    t2.append(tt)
                sim = psum_pool.tile([128, w], BF16, tag="sim",
                                     name=f"sim{ci}", bufs=2)
                nc.vector.tensor_tensor(sim[:], t2[0][:], t2[1][:], ALU.add)

                # positives (pre-exp) from chunk B0 cols [0,128)
                if space == "B" and lo == 0:
                    scrP = pacc.tile([128, 128], BF16, tag="scrP")
                    possum = plo.tile([128, 1], F32, tag="possum")
                    nc.vector.scalar_tensor_tensor(
                        scrP[:], sim[:, 0:128], 1.0, eyeb[:],
                        ALU.mult, ALU.mult, accum_out=possum[:])

                ex = psum_pool.tile([128, w], BF16, tag="ex", name=f"ex{ci}",
                                    bufs=5)
                rs = plo.tile([128, 1], F32, tag=f"rs{ci}")
                nc.scalar.activation(ex[:], sim[:], AF.Exp, scale=INV_T_S2,
                                     accum_out=rs[:])
                rs_parts.append((space, rs))

                # eye-masked diag removal (cc1 chunk 0)
                if space == "A" and lo == 0:
                    scrA = pacc.tile([128, 128], BF16, tag="scrA")
                    expdA = plo.tile([128, 1], F32, tag="expdA")
                    nc.vector.scalar_tensor_tensor(
                        scrA[:], ex[:, 0:128], 1.0, eyeb[:],
                        ALU.mult, ALU.mult, accum_out=expdA[:])
                if space == "B":
                    scr0 = pacc.tile([128, w], BF16, tag="scr0",
                                     name=f"scr0_{ci}", bufs=2)
                    ra = plo.tile([128, 1], F32, tag=f"ra{ci}")
                    nc.vector.scalar_tensor_tensor(
                        scr0[:], ex[:], 1.0, mskb[:, lo:hi],
                        ALU.mult, ALU.mult, accum_out=ra[:])
                    rsA2_parts.append(ra)
                    scr1 = pacc.tile([128, w], BF16, tag="scr1",
                                     name=f"scr1_{ci}", bufs=2)
                    rb = plo.tile([128, 1], F32, tag=f"rb{ci}")
                    nc.vector.scalar_tensor_tensor(
                        scr1[:], ex[:], 1.0, mskb[:, CH2 + lo:CH2 + hi],
                        ALU.mult, ALU.mult, accum_out=rb[:])
                    expdB_parts.append(rb)

                # column sums (skip cc1 diag block j=0)
                blocks = list(range(w // 128))
                if space == "A" and lo == 0:
                    blocks = blocks[1:]
                if blocks:
                    nb = len(blocks)
                    ps4 = pps.tile([128, nb], F32, tag="ps",
                                   name=f"pc{ci}")
                    for bi, j in enumerate(blocks):
                        nc.tensor.matmul(ps4[:, bi:bi + 1],
                                         ex[:, 128 * j:128 * (j + 1)],
                                         ones1[:], start=True, stop=True)
                    nc.scalar.activation(pc_sb[:, pc_idx:pc_idx + nb],
                                         ps4[:], AF.Copy)
                    pc_idx += nb

            # ---- combine denominators ----
            denA = plo.tile([128, 1], F32, tag="denA")
            denB = plo.tile([128, 1], F32, tag="denB")
            rsA1 = plo.tile([128, 1], F32, tag="rsA1")
            a_parts = [r for sp, r in rs_parts if sp == "A"]
            b_parts = [r for sp, r in rs_parts if sp == "B"]
            nc.vector.tensor_tensor(rsA1[:], a_parts[0][:], a_parts[1][:],
                                    ALU.add)
            rs2t = plo.tile([128, 1], F32, tag="rs2t")
            nc.vector.tensor_tensor(rs2t[:], b_parts[0][:], b_parts[1][:],
                                    ALU.add)
            nc.vector.tensor_tensor(rs2t[:], rs2t[:], b_parts[2][:], ALU.add)
            rsA2 = plo.tile([128, 1], F32, tag="rsA2")
            nc.vector.tensor_tensor(rsA2[:], rsA2_parts[0][:],
                                    rsA2_parts[1][:], ALU.add)
            nc.vector.tensor_tensor(rsA2[:], rsA2[:], rsA2_parts[2][:],
                                    ALU.add)
            expdB = plo.tile([128, 1], F32, tag="expdB")
            nc.vector.tensor_tensor(expdB[:], expdB_parts[0][:],
                                    expdB_parts[1][:], ALU.add)
            nc.vector.tensor_tensor(expdB[:], expdB[:], expdB_parts[2][:],
                                    ALU.add)
            nc.vector.tensor_tensor(denA[:], rsA1[:], rsA2[:], ALU.add)
            nc.vector.tensor_sub(denA[:], denA[:], expdA[:])
            nc.vector.tensor_sub(denB[:], rs2t[:], rsA2[:])
            nc.vector.tensor_sub(denB[:], denB[:], expdB[:])

            # colsum redistribution via transpose + rt
            ps_t = pps.tile([128, 128], BF16, tag="ps", name="ps_t")
            nc.tensor.transpose(ps_t[0:16, :], pc_sb[:], eyeb[:])
            pt_sb = plo.tile([16, 128], BF16, tag="pt_sb")
            nc.vector.tensor_copy(pt_sb[:], ps_t[0:16, :])
            ps_add = pps.tile([128, 16], F32, tag="ps", name="ps_add")
            nc.tensor.matmul(ps_add[:], pt_sb[:], rtb[:], start=True,
                             stop=True)

            den16 = plo.tile([128, 16], F32, tag="den16")
            nc.vector.scalar_tensor_tensor(
                den16[:], cmb[:, 0:16], 1.0,
                denA[:].to_broadcast((128, 16)), ALU.mult, ALU.mult)
            t2m = plo.tile([128, 16], F32, tag="t2m")
            nc.vector.scalar_tensor_tensor(
                t2m[:], cmb[:, 16:32], 1.0,
                denB[:].to_broadcast((128, 16)), ALU.mult, ALU.mult)
            nc.vector.tensor_tensor(den16[:], den16[:], t2m[:], ALU.add)
            nc.vector.tensor_tensor(den16[:], den16[:], ps_add[:], ALU.add)

            nc.sync.dma_start(ccd_in[:], den16[:])
            nc.gpsimd.collective_compute(
                "AllToAll", ALU.bypass, replica_groups=grp,
                ins=[ccd_in[:].opt()], outs=[ccd_out[:].opt()])

            # hide the natural_log table load under the A2A
            junk3 = pers.tile([128, 16], F32, tag="junk3")
            nc.scalar.activation(junk3[:], junk[:], AF.Ln)

            dld = plo.tile([16, 8 * 16], F32, tag="dld")
            for s in range(8):
                nc.sync.dma_start(dld[:, 16 * s:16 * (s + 1)],
                                  ccd_out[16 * s:16 * (s + 1), :])
            dv = dld[:].rearrange("p (s w) -> p s w", s=8)
            d4 = []
            for a in range(4):
                tt = plo.tile([16, 16], F32, tag=f"d4_{a}")
                nc.vector.tensor_tensor(tt[:], dv[:, 2 * a, :],
                                        dv[:, 2 * a + 1, :], ALU.add)
                d4.append(tt)
            d2 = []
            for a in range(2):
                tt = plo.tile([16, 16], F32, tag=f"d2_{a}")
                nc.vector.tensor_tensor(tt[:], d4[2 * a][:],
                                        d4[2 * a + 1][:], ALU.add)
                d2.append(tt)
            denf = plo.tile([16, 16], F32, tag="denf")
            nc.vector.tensor_tensor(denf[:], d2[0][:], d2[1][:], ALU.add)

            lnj = plo.tile([16, 16], F32, tag="lnj")
            lnacc = plo.tile([16, 1], F32, tag="lnacc")
            nc.scalar.activation(lnj[:], denf[:], AF.Ln, accum_out=lnacc[:])

            loss_ps = pps.tile([1, 1], F32, tag="ps", name="loss")
            nc.tensor.matmul(loss_ps[:], lnacc[:], onesf[0:16, :],
                             start=True, stop=False)
            nc.tensor.matmul(loss_ps[:], possum[:], negf2[:],
                             start=False, stop=True)
            out_sb = pers.tile([1, 1], F32, tag="outsb")
            nc.vector.tensor_copy(out_sb[:], loss_ps[:])
            nc.sync.dma_start(y[:], out_sb[:])

    nc.compile()
    _CACHE["nc"] = nc
    return nc


def _make_inputs(emb_i, emb_j):
    e = np.concatenate([np.asarray(emb_i, np.float32),
                        np.asarray(emb_j, np.float32)], axis=0)
    sel = np.zeros((128, 2, 128), np.float32)
    for p in range(128):
        sel[p, :, np.arange(p % 16, 128, 16)] = 1.0
    sel = sel.reshape(128, 256).astype(ml_dtypes.float8_e4m3)
    eye = np.eye(128, dtype=np.float32).astype(ml_dtypes.bfloat16)

    in_maps = []
    for c in range(NCORES):
        loc = e[:, :, 16 * c:16 * (c + 1)]
        t = loc.reshape(R, 8, 8, 2, 16)
        t = t.transpose(1, 2, 4, 3, 0)
        x = np.ascontiguousarray(t).reshape(KT, 128, 2 * R).astype(
            ml_dtypes.float8_e4m3)

        wa2 = 1024 - 128 * c
        msk = np.zeros((2, 128, CH2), np.float32)
        msk[0, :, 0:wa2] = 1.0
        jd = 8 - c
        msk[1, np.arange(128), 128 * jd + np.arange(128)] = 1.0

        rt_m = np.zeros((16, 16), np.float32)
        for col in range(16):
            if col < 15 - c:
                rt_m[col, c + 1 + col] = 1.0
            elif col >= 16 - c:
                rt_m[col, col] = 1.0
        cm_m = np.zeros((2, 128, 16), np.float32)
        cm_m[0, :, c] = 1.0
        cm_m[1, :, 15 - c] = 1.0

        in_maps.append({
            "x": x, "sel": sel, "eye": eye,
            "msk": msk.astype(ml_dtypes.bfloat16),
            "rt": rt_m.astype(ml_dtypes.bfloat16),
            "cm": cm_m.astype(np.float32),
        })
    return in_maps


def run(emb_i, emb_j, **spmd_kwargs):
    nc = _build_nc()
    in_maps = _make_inputs(emb_i, emb_j)
    res = bass_utils.run_bass_kernel_spmd(
        nc, in_maps, core_ids=list(range(NCORES)), **spmd_kwargs)
    total = sum(float(r["y"][0, 0]) for r in res.results)
    return np.array(total / R, dtype=np.float32), res


def kernel(emb_i, emb_j):
    loss, _ = run(emb_i, emb_j)
    return loss


# revision 18
# speedup vs baseline: 1.1862x; 1.1862x over previous
"""Contrastive loss on 8 TRN2 cores — v5 (A2A-pipelined gram, lean DMA issue).

v4 -> v5: dma_start issue costs ~0.6us serial on the issuing engine's
queue, so DMA count is minimized (wide ship/load tiles, 2 DMAs per chunk)
and ship/load DMAs move to the GpSimd queue (which already serializes with
the collective chain); x loads shrink back to 2 DMAs per tile with consts
issued between the first tiles; h0 squares split ACT/DVE so normalize
starts ~22us; final denominator combine is an AllReduce with a per-core
partition mask folded into the closing matmul.
"""

import numpy as np
import ml_dtypes

import concourse.bacc as bacc
import concourse.mybir as mybir
import concourse.tile as tile
from concourse import bass_utils

F32 = mybir.dt.float32
F8E5 = mybir.dt.float8e5
BF16 = mybir.dt.bfloat16
FP8 = mybir.dt.float8e4
AF = mybir.ActivationFunctionType
ALU = mybir.AluOpType
PM = mybir.MatmulPerfMode

B = 1024
R = 2 * B
NCORES = 8
KT = 8
S = 16.0
INV_T_S2 = 2.0 / (S * S)
CH1 = 1024
CH2 = 1152

# column chunks: (space, lo, hi); A = cc1 cols, B = cc2 cols
CHUNKS = [("A", 0, 512), ("A", 512, 1024),
          ("B", 0, 512), ("B", 512, 1024), ("B", 1024, 1152)]

_CACHE = {}


def _pieces(c, lo, hi):
    """Segment pieces of cc2-local cols [lo,hi) for pair c.

    Returns (psum_off, sim_col, width, tile_row) pieces, 512-split."""
    wa2 = 1024 - 128 * c
    tb = 15 - c
    segs = []
    a0, a1 = max(lo, 0), min(hi, wa2)
    if a1 > a0:
        segs.append((a0 - lo, 128 * c + CH1 + a0, a1 - a0, c))
    b0, b1 = max(lo, wa2), min(hi, CH2)
    if b1 > b0:
        segs.append((b0 - lo, 128 * tb + (b0 - wa2), b1 - b0, tb))
    out = []
    for po, sc, w, tr in segs:
        off = 0
        while off < w:
            ww = min(512, w - off)
            out.append((po + off, sc + off, ww, tr))
            off += ww
    return out


def _chunk_units(space, lo, hi):
    units = []
    for u in range(8):
        if space == "A":
            units.append([(0, 128 * u + lo, hi - lo, u)])
        else:
            units.append(_pieces(u, lo, hi))
    return units


def _build_nc():
    if "nc" in _CACHE:
        return _CACHE["nc"]
    nc = bacc.Bacc("TRN2", target_bir_lowering=False, debug=False,
                   num_devices=NCORES)

    x = nc.dram_tensor("x", [KT, 128, 2 * R], FP8, kind="ExternalInput")
    sel = nc.dram_tensor("sel", [128, 256], FP8, kind="ExternalInput")
    eye = nc.dram_tensor("eye", [128, 128], BF16, kind="ExternalInput")
    msk = nc.dram_tensor("msk", [2, 128, CH2], BF16, kind="ExternalInput")
    rt = nc.dram_tensor("rt", [16, 16], BF16, kind="ExternalInput")
    cm = nc.dram_tensor("cm", [2, 128, 16], F32, kind="ExternalInput")
    msel = nc.dram_tensor("msel", [128, 1], F32, kind="ExternalInput")
    y = nc.dram_tensor("y", [1, 1], F32, kind="ExternalOutput")

    cc = []
    for ci, (space, lo, hi) in enumerate(CHUNKS):
        w = hi - lo
        cin = nc.dram_tensor(f"cc{ci}_in", [1024, w], F8E5)
        cout = nc.dram_tensor(f"cc{ci}_out", [1024, w], F8E5)
        cc.append((cin, cout))
    ccd_in = nc.dram_tensor("ccd_in", [128, 16], F32)
    ccd_out = nc.dram_tensor("ccd_out", [128, 16], F32)
    wu_in = nc.dram_tensor("wu_in", [16, 16], F8E5)
    wu_out = nc.dram_tensor("wu_out", [16, 16], F8E5)
    grp = [list(range(NCORES))]

    with tile.TileContext(nc) as tc:
        with tc.tile_pool(name="x8", bufs=KT) as px8, \
             tc.tile_pool(name="sq", bufs=4) as psq, \
             tc.tile_pool(name="pers", bufs=1) as pers, \
             tc.tile_pool(name="simsb", bufs=2) as psim, \
             tc.tile_pool(name="acc", bufs=2) as pacc, \
             tc.tile_pool(name="sum", bufs=2) as psum_pool, \
             tc.tile_pool(name="loss", bufs=1) as plo, \
             tc.tile_pool(name="ps", bufs=7, space="PSUM") as pps:

            # ---- x tiles 0,1 first, then consts, then the rest ----
            xb = []
            for k in range(KT):
                t = px8.tile([128, 2 * R], FP8, tag="x8")
                xb.append(t)

            def load_x(k):
                nc.sync.dma_start(xb[k][0:64, :], x[k, 0:64, :])
                nc.sync.dma_start(xb[k][64:128, :], x[k, 64:128, :])

            load_x(0)
            load_x(1)

            selb = pers.tile([128, 256], FP8, tag="selb")
            nc.sync.dma_start(selb[:], sel[:])
            eyeb = pers.tile([128, 128], BF16, tag="eyeb")
            nc.sync.dma_start(eyeb[:], eye[:])
            mskb = pers.tile([128, 2 * CH2], BF16, tag="mskb")
            nc.sync.dma_start(mskb[:, 0:CH2], msk[0, :, :])
            nc.sync.dma_start(mskb[:, CH2:2 * CH2], msk[1, :, :])
            rtb = pers.tile([16, 16], BF16, tag="rtb")
            nc.sync.dma_start(rtb[:], rt[:])
            cmb = pers.tile([128, 32], F32, tag="cmb")
            nc.sync.dma_start(cmb[:, 0:16], cm[0, :, :])
            nc.sync.dma_start(cmb[:, 16:32], cm[1, :, :])
            mselb = pers.tile([128, 1], F32, tag="mselb")
            nc.sync.dma_start(mselb[:], msel[:])

            for k in range(2, KT):
                load_x(k)

            # ACT table warm: square + abs_reciprocal_sqrt
            junk = pers.tile([128, 16], F32, tag="junk")
            nc.vector.memset(junk[:], 1.0)
            junko = pers.tile([128, 16], F32, tag="junko")
            nc.scalar.activation(junko[:], junk[:], AF.Square)
            nc.scalar.activation(junko[:], junk[:], AF.Abs_reciprocal_sqrt)

            ones1 = pers.tile([128, 1], BF16, tag="ones1")
            nc.vector.memset(ones1[:], 1.0)
            negf2 = pers.tile([128, 1], F32, tag="negf2")
            nc.vector.memset(negf2[:], -2.0 * INV_T_S2)

            # warmup collective on the gpsimd queue
            wub = pers.tile([16, 16], F8E5, tag="wub")
            nc.vector.memset(wub[:], 1.0)
            nc.gpsimd.dma_start(wu_in[:], wub[:])
            nc.gpsimd.collective_compute(
                "AllToAll", ALU.bypass, replica_groups=grp,
                ins=[wu_in[:].opt()], outs=[wu_out[:].opt()])

            selv = selb[:].rearrange("p (two j) -> p two j", two=2)
            scale_t = pers.tile([128, R], FP8, tag="scale_t")

            def vk(k):
                return xb[k][:].rearrange("p (two r) -> p two r", two=2)

            # ---- half h=0: squares split ACT/DVE + ssq matmuls ----
            ssq0 = [pps.tile([128, 512], F32, tag="ps", name=f"ssq0{j}")
                    for j in range(2)]
            for k in range(KT):
                sq = psq.tile([128, 2048], FP8, tag="sq")
                sqv = sq[:].rearrange("p (two r) -> p two r", two=2)
                src = vk(k)[:, :, 0:1024]
                if k % 2 == 0:
                    nc.scalar.activation(sqv, src, AF.Square)
                else:
                    nc.vector.tensor_tensor(sqv, src, src, ALU.mult)
                for j in range(2):
                    nc.tensor.matmul(ssq0[j][:], selv,
                                     sqv[:, :, 512 * j:512 * (j + 1)],
                                     start=(k == 0), stop=(k == KT - 1),
                                     perf_mode=PM.DoubleRow)
            for j in range(2):
                nc.scalar.activation(scale_t[:, 512 * j:512 * (j + 1)],
                                     ssq0[j][:], AF.Abs_reciprocal_sqrt,
                                     scale=128.0 / (S * S))

            def do_norm(h, k):
                for s in range(2):
                    off = s * R + 1024 * h
                    nc.vector.tensor_tensor(
                        xb[k][:, off:off + 1024], xb[k][:, off:off + 1024],
                        scale_t[:, 1024 * h:1024 * h + 1024], ALU.mult)

            # gram chunk machinery -------------------------------------
            chunk_tiles = {}

            def gram_half(ci, half, ks, unit_major=True):
                space, lo, hi = CHUNKS[ci]
                w = hi - lo
                units = _chunk_units(space, lo, hi)
                if ci not in chunk_tiles:
                    chunk_tiles[ci] = [None] * 8
                for ui in range(4 * half, 4 * half + 4):
                    if chunk_tiles[ci][ui] is None:
                        chunk_tiles[ci][ui] = pps.tile(
                            [128, w], F32, tag="ps", name=f"g{ci}_{ui}")
                uis = list(range(4 * half, 4 * half + 4))
                order = ([(ui, k) for ui in uis for k in ks] if unit_major
                         else [(ui, k) for k in ks for ui in uis])
                for ui, k in order:
                    v = vk(k)
                    pt = chunk_tiles[ci][ui]
                    for po, sc, ww, tr in units[ui]:
                        lhsT = v[:, :, 128 * tr:128 * (tr + 1)]
                        nc.tensor.matmul(pt[:, po:po + ww], lhsT,
                                         v[:, :, sc:sc + ww],
                                         start=(k == 0),
                                         stop=(k == KT - 1),
                                         perf_mode=PM.DoubleRow)

            def ship_chunk(ci):
                """ACT-copy chunk psums into one wide fp8e5 tile, 2 DMAs
                to cc_in (gpsimd queue), A2A."""
                space, lo, hi = CHUNKS[ci]
                w = hi - lo
                cin, cout = cc[ci]
                wide = psim.tile([128, 8 * w], F8E5, tag="simsb",
                                 name=f"wide{ci}")
                for ui in range(8):
                    nc.scalar.activation(wide[:, ui * w:(ui + 1) * w],
                                         chunk_tiles[ci][ui][:], AF.Copy)
                wv = wide[:].rearrange("p (u w) -> p u w", u=8)
                cv = cin[:].rearrange("(u p) w -> p u w", u=8)
                nc.gpsimd.dma_start(cv[:, 0:4, :], wv[:, 0:4, :])
                nc.gpsimd.dma_start(cv[:, 4:8, :], wv[:, 4:8, :])
                nc.gpsimd.collective_compute(
                    "AllToAll", ALU.bypass, replica_groups=grp,
                    ins=[cin[:].opt()], outs=[cout[:].opt()])

            # ---- interleave: h1 squares (ACT) + norm h0 (DVE) + gram
            ssq1 = [pps.tile([128, 512], F32, tag="ps", name=f"ssq1{j}")
                    for j in range(2)]
            for k in range(KT):
                sq = psq.tile([128, 2048], FP8, tag="sq")
                sqv = sq[:].rearrange("p (two r) -> p two r", two=2)
                nc.scalar.activation(sqv, vk(k)[:, :, 1024:2048], AF.Square)
                for j in range(2):
                    nc.tensor.matmul(ssq1[j][:], selv,
                                     sqv[:, :, 512 * j:512 * (j + 1)],
                                     start=(k == 0), stop=(k == KT - 1),
                                     perf_mode=PM.DoubleRow)
                do_norm(0, k)
                gram_half(0, 0, [k], unit_major=False)
            for j in range(2):
                off = 1024 + 512 * j
                nc.scalar.activation(scale_t[:, off:off + 512],
                                     ssq1[j][:], AF.Abs_reciprocal_sqrt,
                                     scale=128.0 / (S * S))
            # pull the exp-set table load into the ACT idle window
            nc.scalar.activation(junko[:], junk[:], AF.Exp)

            for k in range(KT):
                do_norm(1, k)
            gram_half(0, 1, list(range(KT)))
            ship_chunk(0)
            for ci in range(1, len(CHUNKS)):
                gram_half(ci, 0, list(range(KT)))
                gram_half(ci, 1, list(range(KT)))
                ship_chunk(ci)

            # ---- per-chunk post-collective loss work ----
            rs_parts = []
            rsA2_parts = []
            expdB_parts = []
            expdA = None
            possum = None
            pc_idx = 0
            pc_sb = plo.tile([128, 16], BF16, tag="pc_sb")

            for ci, (space, lo, hi) in enumerate(CHUNKS):
                w = hi - lo
                cin, cout = cc[ci]
                ld = psum_pool.tile([128, 8 * w], F8E5, tag="ld",
                                    name=f"ld{ci}")
                lv = ld[:].rearrange("p (s w) -> p s w", s=8)
                ov = cout[:].rearrange("(s p) w -> p s w", s=8)
                nc.gpsimd.dma_start(lv[:, 0:4, :], ov[:, 0:4, :])
                nc.gpsimd.dma_start(lv[:, 4:8, :], ov[:, 4:8, :])
                t4 = []
                for a in range(4):
                    tt = psum_pool.tile([128, w], BF16, tag="t4",
                                        name=f"t4_{ci}_{a}", bufs=4)
                    nc.vector.tensor_tensor(tt[:], lv[:, 2 * a, :],
                                            lv[:, 2 * a + 1, :], ALU.add)
                    t4.append(tt)
                t2 = []
                for a in range(2):
                    tt = psum_pool.tile([128, w], BF16, tag="t2",
                                        name=f"t2_{ci}_{a}", bufs=2)
                    nc.vector.tensor_tensor(tt[:], t4[2 * a][:],
                                            t4[2 * a + 1][:], ALU.add)
                    t2.append(tt)
                sim = psum_pool.tile([128, w], BF16, tag="sim",
                                     name=f"sim{ci}", bufs=2)
                nc.vector.tensor_tensor(sim[:], t2[0][:], t2[1][:], ALU.add)

                if space == "B" and lo == 0:
                    scrP = pacc.tile([128, 128], BF16, tag="scrP", bufs=1)
                    possum = plo.tile([128, 1], F32, tag="possum")
                    nc.vector.scalar_tensor_tensor(
                        scrP[:], sim[:, 0:128], 1.0, eyeb[:],
                        ALU.mult, ALU.mult, accum_out=possum[:])

                ex = psum_pool.tile([128, w], BF16, tag="ex", name=f"ex{ci}",
                                    bufs=5)
                rs = plo.tile([128, 1], F32, tag=f"rs{ci}")
                nc.scalar.activation(ex[:], sim[:], AF.Exp, scale=INV_T_S2,
                                     accum_out=rs[:])
                rs_parts.append((space, rs))

                if space == "A" and lo == 0:
                    scrA = pacc.tile([128, 128], BF16, tag="scrA", bufs=1)
                    expdA = plo.tile([128, 1], F32, tag="expdA")
                    nc.vector.scalar_tensor_tensor(
                        scrA[:], ex[:, 0:128], 1.0, eyeb[:],
                        ALU.mult, ALU.mult, accum_out=expdA[:])
                if space == "B":
                    scr0 = pacc.tile([128, w], BF16, tag="scr0",
                                     name=f"scr0_{ci}", bufs=2)
                    ra = plo.tile([128, 1], F32, tag=f"ra{ci}")
                    nc.vector.scalar_tensor_tensor(
                        scr0[:], ex[:], 1.0, mskb[:, lo:hi],
                        ALU.mult, ALU.mult, accum_out=ra[:])
                    rsA2_parts.append(ra)
                    scr1 = pacc.tile([128, w], BF16, tag="scr1",
                                     name=f"scr1_{ci}", bufs=2)
                    rb = plo.tile([128, 1], F32, tag=f"rb{ci}")
                    nc.vector.scalar_tensor_tensor(
                        scr1[:], ex[:], 1.0, mskb[:, CH2 + lo:CH2 + hi],
                        ALU.mult, ALU.mult, accum_out=rb[:])
                    expdB_parts.append(rb)

                # column sums -> pc_sb (transpose + rt matmul at the end)
                blocks = list(range(w // 128))
                if space == "A" and lo == 0:
                    blocks = blocks[1:]
                nb = len(blocks)
                ps4 = pps.tile([128, nb], F32, tag="pc", name=f"pc{ci}",
                               bufs=1)
                for bi, j in enumerate(blocks):
                    nc.tensor.matmul(ps4[:, bi:bi + 1],
                                     ex[:, 128 * j:128 * (j + 1)],
                                     ones1[:], start=True, stop=True)
                nc.scalar.activation(pc_sb[:, pc_idx:pc_idx + nb],
                                     ps4[:], AF.Copy)
                pc_idx += nb

            # colsum redistribution via one end transpose + rt matmul
            ps_t = pps.tile([128, 128], BF16, tag="ps", name="ps_t")
            nc.tensor.transpose(ps_t[0:16, :], pc_sb[:], eyeb[:])
            pt_sb = plo.tile([16, 128], BF16, tag="pt_sb")
            nc.vector.tensor_copy(pt_sb[:], ps_t[0:16, :])
            ps_add = pps.tile([128, 16], F32, tag="ps", name="ps_add")
            nc.tensor.matmul(ps_add[:], pt_sb[:], rtb[:], start=True,
                             stop=True)

            # ---- combine denominators ----
            denA = plo.tile([128, 1], F32, tag="denA")
            denB = plo.tile([128, 1], F32, tag="denB")
            rsA1 = plo.tile([128, 1], F32, tag="rsA1")
            a_parts = [r for sp, r in rs_parts if sp == "A"]
            b_parts = [r for sp, r in rs_parts if sp == "B"]
            nc.vector.tensor_tensor(rsA1[:], a_parts[0][:], a_parts[1][:],
                                    ALU.add)
            rs2t = plo.tile([128, 1], F32, tag="rs2t")
            nc.vector.tensor_tensor(rs2t[:], b_parts[0][:], b_parts[1][:],
                                    ALU.add)
            nc.vector.tensor_tensor(rs2t[:], rs2t[:], b_parts[2][:], ALU.add)
            rsA2 = plo.tile([128, 1], F32, tag="rsA2")
            nc.vector.tensor_tensor(rsA2[:], rsA2_parts[0][:],
                                    rsA2_parts[1][:], ALU.add)
            nc.vector.tensor_tensor(rsA2[:], rsA2[:], rsA2_parts[2][:],
                                    ALU.add)
            expdB = plo.tile([128, 1], F32, tag="expdB")
            nc.vector.tensor_tensor(expdB[:], expdB_parts[0][:],
                                    expdB_parts[1][:], ALU.add)
            nc.vector.tensor_tensor(expdB[:], expdB[:], expdB_parts[2][:],
                                    ALU.add)
            nc.vector.tensor_tensor(denA[:], rsA1[:], rsA2[:], ALU.add)
            nc.vector.tensor_sub(denA[:], denA[:], expdA[:])
            nc.vector.tensor_sub(denB[:], rs2t[:], rsA2[:])
            nc.vector.tensor_sub(denB[:], denB[:], expdB[:])

            den16 = plo.tile([128, 16], F32, tag="den16")
            nc.vector.scalar_tensor_tensor(
                den16[:], cmb[:, 0:16], 1.0,
                denA[:].to_broadcast((128, 16)), ALU.mult, ALU.mult)
            t2m = plo.tile([128, 16], F32, tag="t2m")
            nc.vector.scalar_tensor_tensor(
                t2m[:], cmb[:, 16:32], 1.0,
                denB[:].to_broadcast((128, 16)), ALU.mult, ALU.mult)
            nc.vector.tensor_tensor(den16[:], den16[:], t2m[:], ALU.add)
            nc.vector.tensor_tensor(den16[:], den16[:], ps_add[:], ALU.add)

            nc.gpsimd.dma_start(ccd_in[:], den16[:])
            nc.gpsimd.collective_compute(
                "AllReduce", ALU.add, replica_groups=grp,
                ins=[ccd_in[:].opt()], outs=[ccd_out[:].opt()])

            # hide the natural_log table load under the AllReduce
            junk3 = pers.tile([128, 16], F32, tag="junk3")
            nc.scalar.activation(junk3[:], junk[:], AF.Ln)

            denf = plo.tile([128, 16], F32, tag="denf")
            nc.gpsimd.dma_start(denf[:], ccd_out[:])
            lnj = plo.tile([128, 16], F32, tag="lnj")
            lnacc = plo.tile([128, 1], F32, tag="lnacc")
            nc.scalar.activation(lnj[:], denf[:], AF.Ln, accum_out=lnacc[:])

            loss_ps = pps.tile([1, 1], F32, tag="ps", name="loss_ps")
            nc.tensor.matmul(loss_ps[:], lnacc[:], mselb[:],
                             start=True, stop=False)
            nc.tensor.matmul(loss_ps[:], possum[:], negf2[:],
                             start=False, stop=True)
            out_sb = pers.tile([1, 1], F32, tag="outsb")
            nc.vector.tensor_copy(out_sb[:], loss_ps[:])
            nc.sync.dma_start(y[:], out_sb[:])

    nc.compile()
    _CACHE["nc"] = nc
    return nc


def _make_inputs(emb_i, emb_j):
    e = np.concatenate([np.asarray(emb_i, np.float32),
                        np.asarray(emb_j, np.float32)], axis=0)
    sel = np.zeros((128, 2, 128), np.float32)
    for p in range(128):
        sel[p, :, np.arange(p % 16, 128, 16)] = 1.0
    sel = sel.reshape(128, 256).astype(ml_dtypes.float8_e4m3)
    eye = np.eye(128, dtype=np.float32).astype(ml_dtypes.bfloat16)

    in_maps = []
    for c in range(NCORES):
        loc = e[:, :, 16 * c:16 * (c + 1)]
        t = loc.reshape(R, 8, 8, 2, 16)
        t = t.transpose(1, 2, 4, 3, 0)
        x = np.ascontiguousarray(t).reshape(KT, 128, 2 * R).astype(
            ml_dtypes.float8_e4m3)

        wa2 = 1024 - 128 * c
        msk = np.zeros((2, 128, CH2), np.float32)
        msk[0, :, 0:wa2] = 1.0
        jd = 8 - c
        msk[1, np.arange(128), 128 * jd + np.arange(128)] = 1.0

        rt_m = np.zeros((16, 16), np.float32)
        for col in range(16):
            if col < 15 - c:
                rt_m[col, c + 1 + col] = 1.0
            elif col >= 16 - c:
                rt_m[col, col] = 1.0
        cm_m = np.zeros((2, 128, 16), np.float32)
        cm_m[0, :, c] = 1.0
        cm_m[1, :, 15 - c] = 1.0

        msel_m = np.zeros((128, 1), np.float32)
        msel_m[16 * c:16 * (c + 1), 0] = 1.0

        in_maps.append({
            "x": x, "sel": sel, "eye": eye,
            "msk": msk.astype(ml_dtypes.bfloat16),
            "rt": rt_m.astype(ml_dtypes.bfloat16),
            "cm": cm_m.astype(np.float32),
            "msel": msel_m,
        })
    return in_maps


def run(emb_i, emb_j, **spmd_kwargs):
    nc = _build_nc()
    in_maps = _make_inputs(emb_i, emb_j)
    res = bass_utils.run_bass_kernel_spmd(
        nc, in_maps, core_ids=list(range(NCORES)), **spmd_kwargs)
    total = sum(float(r["y"][0, 0]) for r in res.results)
    return np.array(total / R, dtype=np.float32), res


def kernel(emb_i, emb_j):
    loss, _ = run(emb_i, emb_j)
    return loss
